# revision 28
# baseline (speedup 1.0000x reference)
"""Bayesian uncertainty distance kernel for TRN2 (8 NeuronCores, SPMD).

Math (per reference):
    W_s  = weight_mu + eps_w[s] * softplus(weight_rho)          [S,D,D]
    b_s  = bias_mu   + eps_b[s] * softplus(bias_rho)            [S,D]
    qt_s = query @ W_s + b_s                                    [S,Q,D]
    d2_s = ||qt_s||^2 - 2 qt_s.proto^T + ||proto||^2            [S,Q,P]
    mean = mean_s sqrt(d2_s);  std = std_s(sqrt(d2_s), ddof=1)

Sharding: data-parallel over Q (8192 -> 8 x 1024). Everything else replicated.

On-chip design (per core, Q=1024, P=2048, D=256, S=10):
  - samples are DEFINED as x_s := fp16(-2*(query@W_s + b_s)) so that every
    moment is computed consistently from the same rounded values; first-order
    fp16 rounding error then cancels exactly in the variance.
  - qt matmuls (fp16):   psum = fp16(W_s)^T-block @ fp16(query^T)
  - x_s  = ACT Identity(psum * -2 + (-2 b_s))  -> fp16 SBUF  [e, q] layout
  - x2_s = ACT Square(x_s * 0.5)               -> fp16 (= qt^2)
  - qn_s = matmul(lhsT=x2_s-block, rhs=ones)   -> [128,1] psum column (exact
    free-of-transpose per-partition bias for the sqrt pass)
  - cross: psum = ones x pn16 (rank-1, start) + x_s-block @ proto^T (fp16)
           => psum = -2*cross + pn
  - dist = ACT Sqrt(psum + qn_s bias);  macc += dist (DVE)
  - variance via sum-of-d2:  sum_s d2_s = qnsum + 10*pn + (sum_s x_s).proto^T
    computed with one extra fp32 matmul group from xsum (DVE-accumulated).
    std = sqrt((sum_d2 - macc^2/10)/9 + qnsum/9) via Square/sub/Sqrt.

The host does only O(S*D^2) prep in numpy (softplus, W_s, transposes, pn).
"""

import os
import numpy as np

import concourse.bass as bass
import concourse.mybir as mybir
import concourse.tile as tile
from concourse import bacc, bass_utils

AF = mybir.ActivationFunctionType
ALU = mybir.AluOpType

# Note: walrus's --enable-ldw-opt stays false — fp32 matmuls emit
# InstLdweights that are "not compatible with LDW optimization".
F32 = mybir.dt.float32
F16 = mybir.dt.float16

NCORES = 8
D = 256
Q_FULL = 8192
P = 2048
S = 10
QLOC = Q_FULL // NCORES  # 1024
ET = D // 128  # 2 e-tiles
DT = D // 128  # 2 d-tiles
QT = QLOC // 128  # 8 q-tiles per core
PC = P // 512  # 4 p-chunks
QC = QLOC // 512  # 2 q-chunks

_CACHE = {}
LAST_RESULTS = None


def _build_bass():
    nc = bacc.Bacc(
        "TRN2",
        target_bir_lowering=False,
        debug=False,
        num_devices=NCORES,
    )
    ins = {}
    ins["qT16"] = nc.dram_tensor("qT16", [128, DT * QLOC], F16, kind="ExternalInput").ap()
    ins["W16"] = nc.dram_tensor("W16", [S, 128, DT * 256], F16, kind="ExternalInput").ap()
    ins["b2T"] = nc.dram_tensor("b2T", [128, ET * S], F32, kind="ExternalInput").ap()
    ins["yT16"] = nc.dram_tensor("yT16", [128, ET * P], F16, kind="ExternalInput").ap()
    ins["yT32"] = nc.dram_tensor("yT32", [128, ET * P], F32, kind="ExternalInput").ap()
    ins["yext16"] = nc.dram_tensor("yext16", [2, P], F16, kind="ExternalInput").ap()
    ins["ysext32"] = nc.dram_tensor("ysext32", [2, P], F32, kind="ExternalInput").ap()
    ins["o16c"] = nc.dram_tensor("o16c", [128, 1], F16, kind="ExternalInput").ap()
    ins["eye16"] = nc.dram_tensor("eye16", [128, 128], F16, kind="ExternalInput").ap()
    mean_o = nc.dram_tensor("mean_o", [QLOC, P], F32, kind="ExternalOutput").ap()
    std_o = nc.dram_tensor("std_o", [QLOC, P], F32, kind="ExternalOutput").ap()

    with tile.TileContext(nc) as tc:
        _kernel_body(tc, ins, mean_o, std_o)
    nc.compile()
    return nc


def _kernel_body(tc, ins, mean_o, std_o):
    nc = tc.nc
    from contextlib import ExitStack

    ctx = ExitStack()
    with ctx:
        cpool = ctx.enter_context(tc.tile_pool(name="consts", bufs=1))
        wpool = ctx.enter_context(tc.tile_pool(name="wpool", bufs=2))
        xpool = ctx.enter_context(tc.tile_pool(name="xpool", bufs=S))
        x2pool = ctx.enter_context(tc.tile_pool(name="x2pool", bufs=2))
        xsumpool = ctx.enter_context(tc.tile_pool(name="xsumpool", bufs=1))
        qnpool = ctx.enter_context(tc.tile_pool(name="qnpool", bufs=1))
        distpool = ctx.enter_context(tc.tile_pool(name="distpool", bufs=3))
        maccpool = ctx.enter_context(tc.tile_pool(name="maccpool", bufs=2))
        finpool = ctx.enter_context(tc.tile_pool(name="finpool", bufs=2))
        outpool = ctx.enter_context(tc.tile_pool(name="outpool", bufs=3))
        pp = ctx.enter_context(tc.tile_pool(name="pp", bufs=4, space="PSUM"))

        # ---- constants into SBUF ----
        qT_t = cpool.tile([128, DT * QLOC], F16)
        nc.sync.dma_start(qT_t[:], ins["qT16"])
        b2_t = cpool.tile([128, ET * S], F32)
        nc.sync.dma_start(b2_t[:], ins["b2T"])
        yT16_t = cpool.tile([128, ET * P], F16)
        nc.sync.dma_start(yT16_t[:], ins["yT16"])
        yT32_t = cpool.tile([128, ET * P], F32)
        nc.sync.dma_start(yT32_t[:], ins["yT32"])
        yext16_t = cpool.tile([2, P], F16)
        nc.sync.dma_start(yext16_t[:], ins["yext16"])
        ysext32_t = cpool.tile([2, P], F32)
        nc.sync.dma_start(ysext32_t[:], ins["ysext32"])
        o16c_t = cpool.tile([128, 1], F16)
        nc.sync.dma_start(o16c_t[:], ins["o16c"])
        eye16_t = cpool.tile([128, 128], F16)
        nc.sync.dma_start(eye16_t[:], ins["eye16"])

        xsum_t = xsumpool.tile([128, ET * QLOC], F32)
        # qn rows (fp16, max qn ~55k < 65504): row 0 holds qn for all (s,q),
        # row 1 is ones; [2,128] slices feed the rank-2 (qn+pn) matmul.
        qrow16_t = qnpool.tile([2, S * QLOC], F16)
        nc.vector.memset(qrow16_t[0:2, :], 1.0)
        # ss-side rank-2 operand: row 0 = qnsum (fp32), row 1 = ones
        qsrow32_t = qnpool.tile([2, QLOC], F32)
        nc.vector.memset(qsrow32_t[0:2, :], 1.0)

        x_tiles = []
        # ---------- phase 1: per-sample transformed queries ----------
        for s in range(S):
            w_t = wpool.tile([128, DT * 256], F16, tag="w")
            nc.sync.dma_start(w_t[:], ins["W16"][s])
            x_t = xpool.tile([128, ET * QLOC], F16, tag="x", name=f"x{s}")
            x_tiles.append(x_t)
            x2s = []
            for et in range(ET):
                for qc in range(QC):
                    qp = pp.tile([128, 512], F32, tag="ps", name=f"qp{s}_{et}_{qc}")
                    for dt_ in range(DT):
                        nc.tensor.matmul(
                            qp[:],
                            lhsT=w_t[:, dt_ * 256 + et * 128 : dt_ * 256 + et * 128 + 128],
                            rhs=qT_t[:, dt_ * QLOC + qc * 512 : dt_ * QLOC + qc * 512 + 512],
                            start=(dt_ == 0),
                            stop=(dt_ == DT - 1),
                        )
                    # x = fp16(-2*qt - 2*b) on DVE: (psum * -2) + b2col
                    # (keeps phase-1 ACT light so the PE stream stays dense)
                    nc.vector.tensor_scalar(
                        x_t[:, et * QLOC + qc * 512 : et * QLOC + qc * 512 + 512],
                        qp[:],
                        -2.0,
                        b2_t[:, et * S + s : et * S + s + 1],
                        ALU.mult,
                        ALU.add,
                    )
                x2_t = x2pool.tile([128, QLOC], F16, tag=f"x2_{et}", name=f"x2_{s}_{et}")
                x2s.append(x2_t)
                # x2 = x^2 = 4*qt^2 on ACT (phase 1 is DVE-bound; the 0.25
                # compensation is folded into the qn psum->sbuf copy scale)
                nc.scalar.square(x2_t[:], x_t[:, et * QLOC : (et + 1) * QLOC])
            # qn rows: ones-stationary matmuls (shared lhsT, no LDW tax);
            # 0.25 compensates x2 = (2*qt)^2
            for qc in range(QC):
                qr_p = pp.tile([1, 512], F32, tag="ps", name=f"qr{s}_{qc}")
                for et in range(ET):
                    nc.tensor.matmul(
                        qr_p[:],
                        lhsT=o16c_t[:],
                        rhs=x2s[et][:, qc * 512 : (qc + 1) * 512],
                        start=(et == 0),
                        stop=(et == ET - 1),
                    )
                nc.scalar.mul(
                    qrow16_t[0:1, s * QLOC + qc * 512 : s * QLOC + qc * 512 + 512],
                    qr_p[:],
                    0.25,
                )

        # xsum = sum_s x_s via identity-matmul PSUM accumulation (a mixed
        # fp16+fp32 DVE tensor_tensor measured 13x slower than fp32+fp32,
        # so the PE does the accumulation instead)
        for et in range(ET):
            for qc in range(QC):
                xsp = pp.tile([128, 512], F32, tag="ps", name=f"xsp{et}_{qc}")
                for s in range(S):
                    nc.tensor.matmul(
                        xsp[:],
                        lhsT=eye16_t[:],
                        rhs=x_tiles[s][
                            :, et * QLOC + qc * 512 : et * QLOC + qc * 512 + 512
                        ],
                        start=(s == 0),
                        stop=(s == S - 1),
                    )
                nc.scalar.activation(
                    xsum_t[:, et * QLOC + qc * 512 : et * QLOC + qc * 512 + 512],
                    xsp[:],
                    AF.Copy,
                )

        # qnsum row (fp32) = sum_s of the fp16 qn rows, consistent with the
        # per-sample values the rank-2 matmuls use
        nc.vector.tensor_reduce(
            qsrow32_t[0:1, :],
            qrow16_t[0:1, :].rearrange("p (s q) -> p q s", s=S),
            axis=mybir.AxisListType.X,
            op=ALU.add,
        )

        # ---------- phase 2: distances, moments, outputs ----------
        PH = 1024  # psum tile width (2 banks); 4 bufs deepen the PE pipeline
        NH = P // PH
        for qt_ in range(QT):
            macc_t = maccpool.tile([128, P], F32, tag="macc", name=f"macc{qt_}")
            for s in range(S):
                dist_t = None
                if s > 0:
                    dist_t = distpool.tile([128, P], F32, tag="dist", name=f"d{qt_}_{s}")
                cps = [
                    pp.tile([128, PH], F32, tag="ps", name=f"cp{qt_}_{s}_{h}")
                    for h in range(NH)
                ]
                # lhsT-major ordering: each stationary operand covers all
                # PSUM halves before switching (leader-MM LDW tax once per
                # lhsT instead of once per half)
                lhsT_r2 = qrow16_t[:, s * QLOC + qt_ * 128 : s * QLOC + qt_ * 128 + 128]
                for h in range(NH):
                    for pc in range(PH // 512):
                        o = h * PH + pc * 512
                        nc.tensor.matmul(
                            cps[h][:, pc * 512 : (pc + 1) * 512],
                            lhsT=lhsT_r2,
                            rhs=yext16_t[:, o : o + 512],
                            start=True,
                            stop=False,
                            skip_group_check=True,
                        )
                for et in range(ET):
                    lhs = x_tiles[s][
                        :, et * QLOC + qt_ * 128 : et * QLOC + qt_ * 128 + 128
                    ]
                    for h in range(NH):
                        for pc in range(PH // 512):
                            o = h * PH + pc * 512
                            nc.tensor.matmul(
                                cps[h][:, pc * 512 : (pc + 1) * 512],
                                lhsT=lhs,
                                rhs=yT16_t[:, et * P + o : et * P + o + 512],
                                start=False,
                                stop=(et == ET - 1),
                                skip_group_check=True,
                            )
                dst = macc_t if s == 0 else dist_t
                for h in range(NH):
                    nc.scalar.activation(
                        dst[:, h * PH : (h + 1) * PH], cps[h][:], AF.Sqrt
                    )
                if s > 0:
                    nc.vector.tensor_add(macc_t[:], macc_t[:], dist_t[:])

            # sum_s d2 = qnsum + 10*pn + xsum.proto^T (fp32, rank-2 + cross)
            # m2 = macc^2; u = ssp - m2/10  (all on DVE, ACT stays on sqrt)
            m2_t = finpool.tile([128, P], F32, tag="fin", name=f"m2{qt_}")
            nc.vector.tensor_mul(m2_t[:], macc_t[:], macc_t[:])
            u_t = finpool.tile([128, P], F32, tag="fin", name=f"u{qt_}")
            ssps = [
                pp.tile([128, PH], F32, tag="ps", name=f"ssp{qt_}_{h}")
                for h in range(NH)
            ]
            lhsT_ss = qsrow32_t[:, qt_ * 128 : qt_ * 128 + 128]
            for h in range(NH):
                for pc in range(PH // 512):
                    o = h * PH + pc * 512
                    nc.tensor.matmul(
                        ssps[h][:, pc * 512 : (pc + 1) * 512],
                        lhsT=lhsT_ss,
                        rhs=ysext32_t[:, o : o + 512],
                        start=True,
                        stop=False,
                        skip_group_check=True,
                    )
            for et in range(ET):
                lhs = xsum_t[:, et * QLOC + qt_ * 128 : et * QLOC + qt_ * 128 + 128]
                for h in range(NH):
                    for pc in range(PH // 512):
                        o = h * PH + pc * 512
                        nc.tensor.matmul(
                            ssps[h][:, pc * 512 : (pc + 1) * 512],
                            lhsT=lhs,
                            rhs=yT32_t[:, et * P + o : et * P + o + 512],
                            start=False,
                            stop=(et == ET - 1),
                            skip_group_check=True,
                        )
            for h in range(NH):
                nc.vector.scalar_tensor_tensor(
                    u_t[:, h * PH : (h + 1) * PH],
                    m2_t[:, h * PH : (h + 1) * PH],
                    -1.0 / S,
                    ssps[h][:],
                    ALU.mult,
                    ALU.add,
                )
            ostd_t = outpool.tile([128, P], F32, tag="out", name=f"os{qt_}")
            nc.scalar.activation(ostd_t[:], u_t[:], AF.Sqrt, scale=1.0 / (S - 1))
            omean_t = outpool.tile([128, P], F32, tag="out", name=f"om{qt_}")
            nc.vector.tensor_scalar_mul(omean_t[:], macc_t[:], 1.0 / S)
            nc.sync.dma_start(std_o[qt_ * 128 : (qt_ + 1) * 128, :], ostd_t[:])
            nc.sync.dma_start(mean_o[qt_ * 128 : (qt_ + 1) * 128, :], omean_t[:])


def _prep_inputs(query_features, prototypes, weight_mu, weight_rho, bias_mu, bias_rho, eps_w, eps_b):
    f32, f16 = np.float32, np.float16
    sp_w = np.log1p(np.exp(weight_rho.astype(np.float64))).astype(f32)
    sp_b = np.log1p(np.exp(bias_rho.astype(np.float64))).astype(f32)
    W = (weight_mu[None] + eps_w * sp_w[None]).astype(f32)  # [S,D,D]
    B = (bias_mu[None] + eps_b * sp_b[None]).astype(f32)  # [S,D]
    Wh = W.astype(f16)
    qfh = query_features.astype(f16)  # [Q,D]
    yh = prototypes.astype(f16)  # [P,D]
    pn = (yh.astype(f32) ** 2).sum(-1, dtype=f32)  # [P]
    pn16 = pn.astype(f16)
    pn10 = (float(S) * pn16.astype(f32)).astype(f32)
    b2 = (-2.0 * B).astype(f32)  # [S,D]

    W16 = np.ascontiguousarray(
        Wh.reshape(S, DT, 128, 256).transpose(0, 2, 1, 3).reshape(S, 128, DT * 256)
    )
    b2T = np.ascontiguousarray(
        b2.T.reshape(ET, 128, S).transpose(1, 0, 2).reshape(128, ET * S)
    )
    yT16 = np.ascontiguousarray(
        yh.T.reshape(ET, 128, P).transpose(1, 0, 2).reshape(128, ET * P)
    )
    yT32 = yT16.astype(f32)
    yext16 = np.stack([np.ones(P, f16), pn16]).astype(f16)  # [2,P]
    ysext32 = np.stack([np.ones(P, f32), pn10]).astype(f32)  # [2,P]
    common = {
        "W16": W16,
        "b2T": b2T,
        "yT16": yT16,
        "yT32": yT32,
        "yext16": yext16,
        "ysext32": ysext32,
        "o16c": np.ones((128, 1), f16),
        "eye16": np.eye(128, dtype=f16),
    }
    in_maps = []
    for c in range(NCORES):
        qs = qfh[c * QLOC : (c + 1) * QLOC]  # [QLOC, D]
        qT16 = np.ascontiguousarray(
            qs.T.reshape(DT, 128, QLOC).transpose(1, 0, 2).reshape(128, DT * QLOC)
        )
        in_maps.append({"qT16": qT16, **common})
    return in_maps


def kernel(**inputs):
    global LAST_RESULTS
    n_samples = int(inputs.pop("n_samples", S))
    assert n_samples == S, f"kernel hardcodes S={S}, got {n_samples}"
    np_inputs = {
        k: np.asarray(v, dtype=np.float32)
        for k, v in inputs.items()
    }
    in_maps = _prep_inputs(**np_inputs)

    if "nc" not in _CACHE:
        _CACHE["nc"] = _build_bass()
    nc = _CACHE["nc"]

    trace = bool(int(os.environ.get("KERNEL_TRACE", "0")))
    res = bass_utils.run_bass_kernel_spmd(
        nc, in_maps, core_ids=list(range(NCORES)), trace=trace
    )
    LAST_RESULTS = res
    mean = np.concatenate([r["mean_o"] for r in res.results], axis=0)
    std = np.concatenate([r["std_o"] for r in res.results], axis=0)
    return mean, std


# revision 32
# speedup vs baseline: 1.0146x; 1.0146x over previous
"""Bayesian uncertainty distance kernel for TRN2 (8 NeuronCores, SPMD).

Math (per reference):
    W_s  = weight_mu + eps_w[s] * softplus(weight_rho)          [S,D,D]
    b_s  = bias_mu   + eps_b[s] * softplus(bias_rho)            [S,D]
    qt_s = query @ W_s + b_s                                    [S,Q,D]
    d2_s = ||qt_s||^2 - 2 qt_s.proto^T + ||proto||^2            [S,Q,P]
    mean = mean_s sqrt(d2_s);  std = std_s(sqrt(d2_s), ddof=1)

Sharding: data-parallel over Q (8192 -> 8 x 1024). Everything else replicated.

On-chip design (per core, Q=1024, P=2048, D=256, S=10), ~405us measured:
  - samples are DEFINED as x_s := fp16(-2*(query@W_s + b_s)) so that every
    moment is computed consistently from the same rounded values; first-order
    fp16 rounding error then cancels exactly in the variance (an inconsistent
    16-bit path measured 27% std error from catastrophic cancellation in
    E[d^2]-E[d]^2; this consistent one measures ~1.6e-3).
  - phase 1 per s: fp16 qt matmuls (W_s stationary, query^T moving) ->
    x_s = DVE tensor_scalar(psum*-2 + (-2 b_s)) -> fp16 [e,q] SBUF;
    x2 = ACT Square(x_s); qn rows = ones-stationary matmul of x2
    (scale 0.25 on the psum->sbuf copy), stored as fp16 rows with a
    companion all-ones row for the rank-2 update below.
  - xsum = sum_s x_s via identity-matmul PSUM accumulation (mixed-dtype
    DVE tensor_tensor measured 13x slow); qnsum row = DVE reduce of qn rows.
  - phase 2 per (qtile, s): PSUM d2 = rank-2([qn_s;1] x [1;pn]) +
    x_s-block @ proto^T (fp16, K=2x128, lhsT-major order to minimize
    LDWEIGHTS boundaries); dist = ACT Sqrt(psum), no bias needed;
    macc += dist (DVE fp32).
  - variance via sum-of-d2: ss = rank-2([qnsum;1] x [1;10*pn]) +
    xsum @ proto^T in fp32; u = ss - macc^2/10 (DVE); std = Sqrt(u/9).
  - mean = macc/10 on DVE (gpsimd tensor_scalar measured 29us/tile).

The host does only O(S*D^2) prep in numpy (softplus, W_s, transposes, pn).
"""

import os
import numpy as np

import concourse.bass as bass
import concourse.mybir as mybir
import concourse.tile as tile
from concourse import bacc, bass_utils

AF = mybir.ActivationFunctionType
ALU = mybir.AluOpType

# Note: walrus's --enable-ldw-opt stays false — fp32 matmuls emit
# InstLdweights that are "not compatible with LDW optimization".
F32 = mybir.dt.float32
F16 = mybir.dt.float16

NCORES = 8
D = 256
Q_FULL = 8192
P = 2048
S = 10
QLOC = Q_FULL // NCORES  # 1024
ET = D // 128  # 2 e-tiles
DT = D // 128  # 2 d-tiles
QT = QLOC // 128  # 8 q-tiles per core
PC = P // 512  # 4 p-chunks
QC = QLOC // 512  # 2 q-chunks

_CACHE = {}
LAST_RESULTS = None


def _build_bass():
    nc = bacc.Bacc(
        "TRN2",
        target_bir_lowering=False,
        debug=False,
        num_devices=NCORES,
    )
    ins = {}
    ins["qT16"] = nc.dram_tensor("qT16", [128, DT * QLOC], F16, kind="ExternalInput").ap()
    ins["W16"] = nc.dram_tensor("W16", [S, 128, DT * 256], F16, kind="ExternalInput").ap()
    ins["b2T"] = nc.dram_tensor("b2T", [128, ET * S], F32, kind="ExternalInput").ap()
    ins["yT16"] = nc.dram_tensor("yT16", [128, ET * P], F16, kind="ExternalInput").ap()
    ins["yT32"] = nc.dram_tensor("yT32", [128, ET * P], F32, kind="ExternalInput").ap()
    ins["yext16"] = nc.dram_tensor("yext16", [2, P], F16, kind="ExternalInput").ap()
    ins["ysext32"] = nc.dram_tensor("ysext32", [2, P], F32, kind="ExternalInput").ap()
    ins["o16c"] = nc.dram_tensor("o16c", [128, 1], F16, kind="ExternalInput").ap()
    ins["eye16"] = nc.dram_tensor("eye16", [128, 128], F16, kind="ExternalInput").ap()
    mean_o = nc.dram_tensor("mean_o", [QLOC, P], F32, kind="ExternalOutput").ap()
    std_o = nc.dram_tensor("std_o", [QLOC, P], F32, kind="ExternalOutput").ap()

    with tile.TileContext(nc) as tc:
        _kernel_body(tc, ins, mean_o, std_o)
    nc.compile()
    return nc


def _kernel_body(tc, ins, mean_o, std_o):
    nc = tc.nc
    from contextlib import ExitStack

    ctx = ExitStack()
    with ctx:
        cpool = ctx.enter_context(tc.tile_pool(name="consts", bufs=1))
        wpool = ctx.enter_context(tc.tile_pool(name="wpool", bufs=2))
        xpool = ctx.enter_context(tc.tile_pool(name="xpool", bufs=S))
        x2pool = ctx.enter_context(tc.tile_pool(name="x2pool", bufs=2))
        xsumpool = ctx.enter_context(tc.tile_pool(name="xsumpool", bufs=1))
        qnpool = ctx.enter_context(tc.tile_pool(name="qnpool", bufs=1))
        distpool = ctx.enter_context(tc.tile_pool(name="distpool", bufs=3))
        maccpool = ctx.enter_context(tc.tile_pool(name="maccpool", bufs=2))
        finpool = ctx.enter_context(tc.tile_pool(name="finpool", bufs=2))
        outpool = ctx.enter_context(tc.tile_pool(name="outpool", bufs=3))
        pp = ctx.enter_context(tc.tile_pool(name="pp", bufs=4, space="PSUM"))

        # ---- constants into SBUF ----
        qT_t = cpool.tile([128, DT * QLOC], F16)
        nc.sync.dma_start(qT_t[:], ins["qT16"])
        b2_t = cpool.tile([128, ET * S], F32)
        nc.sync.dma_start(b2_t[:], ins["b2T"])
        yT16_t = cpool.tile([128, ET * P], F16)
        nc.sync.dma_start(yT16_t[:], ins["yT16"])
        yT32_t = cpool.tile([128, ET * P], F32)
        nc.sync.dma_start(yT32_t[:], ins["yT32"])
        yext16_t = cpool.tile([2, P], F16)
        nc.sync.dma_start(yext16_t[:], ins["yext16"])
        ysext32_t = cpool.tile([2, P], F32)
        nc.sync.dma_start(ysext32_t[:], ins["ysext32"])
        o16c_t = cpool.tile([128, 1], F16)
        nc.sync.dma_start(o16c_t[:], ins["o16c"])
        eye16_t = cpool.tile([128, 128], F16)
        nc.sync.dma_start(eye16_t[:], ins["eye16"])

        xsum_t = xsumpool.tile([128, ET * QLOC], F32)
        # qn rows (fp16, max qn ~55k < 65504): row 0 holds qn for all (s,q),
        # row 1 is ones; [2,128] slices feed the rank-2 (qn+pn) matmul.
        qrow16_t = qnpool.tile([2, S * QLOC], F16)
        nc.vector.memset(qrow16_t[0:2, :], 1.0)
        # ss-side rank-2 operand: row 0 = qnsum (fp32), row 1 = ones
        qsrow32_t = qnpool.tile([2, QLOC], F32)
        nc.vector.memset(qsrow32_t[0:2, :], 1.0)

        x_tiles = []
        # ---------- phase 1: per-sample transformed queries ----------
        for s in range(S):
            w_t = wpool.tile([128, DT * 256], F16, tag="w")
            nc.sync.dma_start(w_t[:], ins["W16"][s])
            x_t = xpool.tile([128, ET * QLOC], F16, tag="x", name=f"x{s}")
            x_tiles.append(x_t)
            x2s = []
            for et in range(ET):
                for qc in range(QC):
                    qp = pp.tile([128, 512], F32, tag="ps", name=f"qp{s}_{et}_{qc}")
                    for dt_ in range(DT):
                        nc.tensor.matmul(
                            qp[:],
                            lhsT=w_t[:, dt_ * 256 + et * 128 : dt_ * 256 + et * 128 + 128],
                            rhs=qT_t[:, dt_ * QLOC + qc * 512 : dt_ * QLOC + qc * 512 + 512],
                            start=(dt_ == 0),
                            stop=(dt_ == DT - 1),
                        )
                    # x = fp16(-2*qt - 2*b) on DVE: (psum * -2) + b2col
                    # (keeps phase-1 ACT light so the PE stream stays dense)
                    nc.vector.tensor_scalar(
                        x_t[:, et * QLOC + qc * 512 : et * QLOC + qc * 512 + 512],
                        qp[:],
                        -2.0,
                        b2_t[:, et * S + s : et * S + s + 1],
                        ALU.mult,
                        ALU.add,
                    )
                x2_t = x2pool.tile([128, QLOC], F16, tag=f"x2_{et}", name=f"x2_{s}_{et}")
                x2s.append(x2_t)
                # x2 = x^2 = 4*qt^2 on ACT (phase 1 is DVE-bound; the 0.25
                # compensation is folded into the qn psum->sbuf copy scale)
                nc.scalar.square(x2_t[:], x_t[:, et * QLOC : (et + 1) * QLOC])
            # qn rows: ones-stationary matmuls (shared lhsT, no LDW tax);
            # 0.25 compensates x2 = (2*qt)^2
            for qc in range(QC):
                qr_p = pp.tile([1, 512], F32, tag="ps", name=f"qr{s}_{qc}")
                for et in range(ET):
                    nc.tensor.matmul(
                        qr_p[:],
                        lhsT=o16c_t[:],
                        rhs=x2s[et][:, qc * 512 : (qc + 1) * 512],
                        start=(et == 0),
                        stop=(et == ET - 1),
                    )
                nc.scalar.mul(
                    qrow16_t[0:1, s * QLOC + qc * 512 : s * QLOC + qc * 512 + 512],
                    qr_p[:],
                    0.25,
                )

        # xsum = sum_s x_s via identity-matmul PSUM accumulation (a mixed
        # fp16+fp32 DVE tensor_tensor measured 13x slower than fp32+fp32,
        # so the PE does the accumulation instead)
        for et in range(ET):
            for qc in range(QC):
                xsp = pp.tile([128, 512], F32, tag="ps", name=f"xsp{et}_{qc}")
                for s in range(S):
                    nc.tensor.matmul(
                        xsp[:],
                        lhsT=eye16_t[:],
                        rhs=x_tiles[s][
                            :, et * QLOC + qc * 512 : et * QLOC + qc * 512 + 512
                        ],
                        start=(s == 0),
                        stop=(s == S - 1),
                    )
                nc.scalar.activation(
                    xsum_t[:, et * QLOC + qc * 512 : et * QLOC + qc * 512 + 512],
                    xsp[:],
                    AF.Copy,
                )

        # qnsum row (fp32) = sum_s of the fp16 qn rows, consistent with the
        # per-sample values the rank-2 matmuls use
        nc.vector.tensor_reduce(
            qsrow32_t[0:1, :],
            qrow16_t[0:1, :].rearrange("p (s q) -> p q s", s=S),
            axis=mybir.AxisListType.X,
            op=ALU.add,
        )

        # ---------- phase 2: distances, moments, outputs ----------
        PH = 1024  # psum tile width (2 banks); 4 bufs deepen the PE pipeline
        NH = P // PH
        for qt_ in range(QT):
            macc_t = maccpool.tile([128, P], F32, tag="macc", name=f"macc{qt_}")
            for s in range(S):
                dist_t = None
                if s > 0:
                    dist_t = distpool.tile([128, P], F32, tag="dist", name=f"d{qt_}_{s}")
                cps = [
                    pp.tile([128, PH], F32, tag="ps", name=f"cp{qt_}_{s}_{h}")
                    for h in range(NH)
                ]
                # lhsT-major ordering: each stationary operand covers all
                # PSUM halves before switching (leader-MM LDW tax once per
                # lhsT instead of once per half)
                lhsT_r2 = qrow16_t[:, s * QLOC + qt_ * 128 : s * QLOC + qt_ * 128 + 128]
                for h in range(NH):
                    for pc in range(PH // 512):
                        o = h * PH + pc * 512
                        nc.tensor.matmul(
                            cps[h][:, pc * 512 : (pc + 1) * 512],
                            lhsT=lhsT_r2,
                            rhs=yext16_t[:, o : o + 512],
                            start=True,
                            stop=False,
                            skip_group_check=True,
                        )
                for et in range(ET):
                    lhs = x_tiles[s][
                        :, et * QLOC + qt_ * 128 : et * QLOC + qt_ * 128 + 128
                    ]
                    for h in range(NH):
                        for pc in range(PH // 512):
                            o = h * PH + pc * 512
                            nc.tensor.matmul(
                                cps[h][:, pc * 512 : (pc + 1) * 512],
                                lhsT=lhs,
                                rhs=yT16_t[:, et * P + o : et * P + o + 512],
                                start=False,
                                stop=(et == ET - 1),
                                skip_group_check=True,
                            )
                dst = macc_t if s == 0 else dist_t
                for h in range(NH):
                    nc.scalar.activation(
                        dst[:, h * PH : (h + 1) * PH], cps[h][:], AF.Sqrt
                    )
                if s > 0:
                    nc.vector.tensor_add(macc_t[:], macc_t[:], dist_t[:])

            # sum_s d2 = qnsum + 10*pn + xsum.proto^T (fp32, rank-2 + cross)
            # m2 = macc^2; u = ssp - m2/10  (all on DVE, ACT stays on sqrt)
            m2_t = finpool.tile([128, P], F32, tag="fin", name=f"m2{qt_}")
            nc.vector.tensor_mul(m2_t[:], macc_t[:], macc_t[:])
            u_t = finpool.tile([128, P], F32, tag="fin", name=f"u{qt_}")
            ssps = [
                pp.tile([128, PH], F32, tag="ps", name=f"ssp{qt_}_{h}")
                for h in range(NH)
            ]
            lhsT_ss = qsrow32_t[:, qt_ * 128 : qt_ * 128 + 128]
            for h in range(NH):
                for pc in range(PH // 512):
                    o = h * PH + pc * 512
                    nc.tensor.matmul(
                        ssps[h][:, pc * 512 : (pc + 1) * 512],
                        lhsT=lhsT_ss,
                        rhs=ysext32_t[:, o : o + 512],
                        start=True,
                        stop=False,
                        skip_group_check=True,
                    )
            for et in range(ET):
                lhs = xsum_t[:, et * QLOC + qt_ * 128 : et * QLOC + qt_ * 128 + 128]
                for h in range(NH):
                    for pc in range(PH // 512):
                        o = h * PH + pc * 512
                        nc.tensor.matmul(
                            ssps[h][:, pc * 512 : (pc + 1) * 512],
                            lhsT=lhs,
                            rhs=yT32_t[:, et * P + o : et * P + o + 512],
                            start=False,
                            stop=(et == ET - 1),
                            skip_group_check=True,
                        )
            for h in range(NH):
                nc.vector.scalar_tensor_tensor(
                    u_t[:, h * PH : (h + 1) * PH],
                    m2_t[:, h * PH : (h + 1) * PH],
                    -1.0 / S,
                    ssps[h][:],
                    ALU.mult,
                    ALU.add,
                )
            ostd_t = outpool.tile([128, P], F32, tag="out", name=f"os{qt_}")
            nc.scalar.activation(ostd_t[:], u_t[:], AF.Sqrt, scale=1.0 / (S - 1))
            omean_t = outpool.tile([128, P], F32, tag="out", name=f"om{qt_}")
            nc.vector.tensor_scalar_mul(omean_t[:], macc_t[:], 1.0 / S)
            nc.sync.dma_start(std_o[qt_ * 128 : (qt_ + 1) * 128, :], ostd_t[:])
            nc.sync.dma_start(mean_o[qt_ * 128 : (qt_ + 1) * 128, :], omean_t[:])


def _prep_inputs(query_features, prototypes, weight_mu, weight_rho, bias_mu, bias_rho, eps_w, eps_b):
    f32, f16 = np.float32, np.float16
    sp_w = np.log1p(np.exp(weight_rho.astype(np.float64))).astype(f32)
    sp_b = np.log1p(np.exp(bias_rho.astype(np.float64))).astype(f32)
    W = (weight_mu[None] + eps_w * sp_w[None]).astype(f32)  # [S,D,D]
    B = (bias_mu[None] + eps_b * sp_b[None]).astype(f32)  # [S,D]
    Wh = W.astype(f16)
    qfh = query_features.astype(f16)  # [Q,D]
    yh = prototypes.astype(f16)  # [P,D]
    pn = (yh.astype(f32) ** 2).sum(-1, dtype=f32)  # [P]
    pn16 = pn.astype(f16)
    pn10 = (float(S) * pn16.astype(f32)).astype(f32)
    b2 = (-2.0 * B).astype(f32)  # [S,D]

    W16 = np.ascontiguousarray(
        Wh.reshape(S, DT, 128, 256).transpose(0, 2, 1, 3).reshape(S, 128, DT * 256)
    )
    b2T = np.ascontiguousarray(
        b2.T.reshape(ET, 128, S).transpose(1, 0, 2).reshape(128, ET * S)
    )
    yT16 = np.ascontiguousarray(
        yh.T.reshape(ET, 128, P).transpose(1, 0, 2).reshape(128, ET * P)
    )
    yT32 = yT16.astype(f32)
    yext16 = np.stack([np.ones(P, f16), pn16]).astype(f16)  # [2,P]
    ysext32 = np.stack([np.ones(P, f32), pn10]).astype(f32)  # [2,P]
    common = {
        "W16": W16,
        "b2T": b2T,
        "yT16": yT16,
        "yT32": yT32,
        "yext16": yext16,
        "ysext32": ysext32,
        "o16c": np.ones((128, 1), f16),
        "eye16": np.eye(128, dtype=f16),
    }
    in_maps = []
    for c in range(NCORES):
        qs = qfh[c * QLOC : (c + 1) * QLOC]  # [QLOC, D]
        qT16 = np.ascontiguousarray(
            qs.T.reshape(DT, 128, QLOC).transpose(1, 0, 2).reshape(128, DT * QLOC)
        )
        in_maps.append({"qT16": qT16, **common})
    return in_maps


def kernel(**inputs):
    global LAST_RESULTS
    n_samples = int(inputs.pop("n_samples", S))
    assert n_samples == S, f"kernel hardcodes S={S}, got {n_samples}"
    np_inputs = {
        k: np.asarray(v, dtype=np.float32)
        for k, v in inputs.items()
    }
    in_maps = _prep_inputs(**np_inputs)

    if "nc" not in _CACHE:
        _CACHE["nc"] = _build_bass()
    nc = _CACHE["nc"]

    trace = bool(int(os.environ.get("KERNEL_TRACE", "0")))
    res = bass_utils.run_bass_kernel_spmd(
        nc, in_maps, core_ids=list(range(NCORES)), trace=trace
    )
    LAST_RESULTS = res
    mean = np.concatenate([r["mean_o"] for r in res.results], axis=0)
    std = np.concatenate([r["std_o"] for r in res.results], axis=0)
    return mean, std
